# revision 14
# baseline (speedup 1.0000x reference)
"""Trainium2 Bass kernel for per-sample segment-mean + tiny GEMM.

Reference computation (per sample b):
    sums[w]  = segment_sum(x[b], word_ids[b])           # [512, 1024]
    cnt[w]   = segment_sum(ones, word_ids[b])           # [512]
    means    = sums / max(cnt, 1)
    out[b]   = means[word_ids[b]] @ W + b               # [2048, 3]

Device strategy (data parallel: 2 samples per core, 8 cores), v4:
  A. Per 128-token chunk: ind[s,w] = (iota[w] == word_id[s]) via DVE
     tensor_scalar(is_equal), issued for BOTH samples up front (consts
     only) so the DVE queue never gates the next sample's matmuls.
     sums[wblock] += ind[:,wblock].T @ x_chunk accumulated in PSUM over
     chunks (word blocks of 128; per-chunk touched word blocks computed
     on host from the sorted ids, unioned across samples so the program
     is SPMD-identical).
  B. Retired sums blocks: scalar-engine evac PSUM->SBUF as fp16, then
     an XBAR DMA transpose (dma_start_transpose, 14ns/16x128-tile)
     produces sumsT[h, hc, w] in one shot, and 8 tiny accumulated PE
     matmuls (lhsT=sumsT slice, rhs=W[h,c] slice) give ysum[w,c] in
     PSUM.  ymean = ysum * (1/max(cnt,1)) via one small DVE op, landing
     in gather-ready [w, c] fp16 layout.  B matmuls are deferred one
     retirement so the transpose never head-blocks the PE queue.
  C. Transposed indicators indT[w,s] per 1024-token strip, built by the
     DVE from a gpsimd partition_broadcast of the ids (wib, fp16), with
     counts accumulated on the fly (accum_out).  outT[c,s] =
     sum_w ymean[w,c] * indT[w,s] (+bias) via accumulated fp16 matmuls.
     Host transposes [3,2048] -> [2048,3] per sample.

  Engine budget per core: DMA ~54us (x stream + transposes, the
  floor), PE ~30us, DVE ~29us, Scalar ~19us, Sync ~13us, GpSimd ~6us.
"""

import numpy as np

import concourse.bass as bass
import concourse.bacc as bacc
import concourse.mybir as mybir
import concourse.tile as tile
from concourse.bass_utils import run_bass_kernel_spmd

B, S, H, C = 16, 2048, 1024, 3
NW = 512
P = 128
N_CORES = 8
SPC = B // N_CORES          # samples per core
NCH = S // P                # 128-token chunks per sample
STRW = 1024                 # gather strip width (tokens)
NST = S // STRW             # strips per sample
NWB = NW // P               # word blocks
NHC = H // P                # h chunks
F32 = mybir.dt.float32
F32R = mybir.dt.float32r
F16 = mybir.dt.float16

_CACHE = {}
TRACE = False          # set by test harness to capture an NTFF profile
LAST_RESULTS = None    # BassKernelResults of the most recent run


def _build_maps(word_ids):
    """Per-chunk / per-strip touched word-block sets, unioned across all
    samples so the same program is valid on every core (untouched blocks
    just accumulate zeros)."""
    chunk_wbs = [set() for _ in range(NCH)]
    strip_wbs = [set() for _ in range(NST)]
    for bi in range(B):
        for ci in range(NCH):
            seg = word_ids[bi, ci * P:(ci + 1) * P]
            lo, hi = int(seg.min()) // P, int(seg.max()) // P
            chunk_wbs[ci].update(range(lo, hi + 1))
        for si in range(NST):
            seg = word_ids[bi, si * STRW:(si + 1) * STRW]
            lo, hi = int(seg.min()) // P, int(seg.max()) // P
            strip_wbs[si].update(range(lo, hi + 1))
    # ensure every word block is touched by at least one chunk so its sums
    # region is always initialized (never-gathered garbage would still
    # poison downstream matmuls as NaN otherwise)
    seen = set().union(*chunk_wbs)
    for wb in range(NWB):
        if wb not in seen:
            chunk_wbs[0].add(wb)
    chunk_wbs = [sorted(sset) for sset in chunk_wbs]
    strip_wbs = [sorted(sset) for sset in strip_wbs]
    first_ch = {wb: min(ci for ci in range(NCH) if wb in chunk_wbs[ci])
                for wb in range(NWB)}
    last_ch = {wb: max(ci for ci in range(NCH) if wb in chunk_wbs[ci])
               for wb in range(NWB)}
    return chunk_wbs, strip_wbs, first_ch, last_ch


def _build_program(maps):
    chunk_wbs, strip_wbs, first_ch, last_ch = maps
    max_nb = max(len(wbs) for wbs in chunk_wbs)
    nc = bacc.Bacc(
        "TRN2",
        target_bir_lowering=False,
        debug=False,
        enable_asserts=False,
        num_devices=N_CORES,
    )

    xc = nc.dram_tensor("xc", [SPC * S, H], F32R, kind="ExternalInput").ap()
    wic = nc.dram_tensor("wic", [P, SPC * NCH], F32, kind="ExternalInput").ap()
    wir = nc.dram_tensor("wir", [1, SPC * S], F16, kind="ExternalInput").ap()
    iota = nc.dram_tensor("iota", [P, NW], F32, kind="ExternalInput").ap()
    iotap = nc.dram_tensor("iotap", [P, NWB], F32, kind="ExternalInput").ap()
    wt = nc.dram_tensor("wt", [P, NHC * C], F16, kind="ExternalInput").ap()
    bb = nc.dram_tensor("bb", [4, 1], F32, kind="ExternalInput").ap()
    yout = nc.dram_tensor("yout", [SPC, C, S], F32, kind="ExternalOutput").ap()

    XCH = 2                     # 128-token chunks per x DMA (1 MB loads)
    XG = NCH // XCH             # x DMA groups per sample

    with tile.TileContext(nc) as tc:
        with (
            tc.tile_pool(name="pp_sums", bufs=2, space="PSUM") as pp_sums,
            tc.tile_pool(name="pp_ymr", bufs=2, space="PSUM") as pp_ymr,
            tc.tile_pool(name="pp_out", bufs=1, space="PSUM") as pp_out,
            tc.tile_pool(name="pl_x", bufs=6) as pl_x,
            tc.tile_pool(name="pl_ind", bufs=2 * NCH) as pl_ind,
            tc.tile_pool(name="pl_sums", bufs=3) as pl_sums,
            tc.tile_pool(name="pl_sumsT", bufs=3) as pl_sumsT,
            tc.tile_pool(name="pl_indT", bufs=10) as pl_indT,
            tc.tile_pool(name="pl_small", bufs=8) as pl_small,
            tc.tile_pool(name="pl_wib", bufs=2) as pl_wib,
            tc.tile_pool(name="pl_out", bufs=2) as pl_out,
            tc.tile_pool(name="pl_const", bufs=1) as pl_const,
        ):
            x_tiles = {}

            def load_x(s, g):
                t = pl_x.tile([P, XCH * H], F32R, tag="x", name=f"x_{s}_{g}")
                r0 = s * S + g * XCH * P
                nc.sync.dma_start(
                    out=t[:].rearrange("p (n h) -> p n h", n=XCH),
                    in_=xc[r0:r0 + XCH * P, :].rearrange(
                        "(n p) h -> p n h", p=P),
                )
                x_tiles[(s, g)] = t

            # prefetch the first x tiles before anything else so the HBM
            # stream starts at t=0
            load_x(0, 0)
            load_x(0, 1)

            # --- constants, loaded once ---
            wic_sb = pl_const.tile([P, SPC * NCH], F32, tag="wic")
            nc.sync.dma_start(out=wic_sb[:], in_=wic[:])
            wir_sb = pl_const.tile([1, SPC * S], F16, tag="wir")
            nc.sync.dma_start(out=wir_sb[:], in_=wir[:])
            iota_sb = pl_const.tile([P, NW], F32, tag="iota")
            nc.sync.dma_start(out=iota_sb[:], in_=iota[:])
            iotap_sb = pl_const.tile([P, NWB], F32, tag="iotap")
            nc.sync.dma_start(out=iotap_sb[:], in_=iotap[:])
            wt_sb = pl_const.tile([P, NHC * C], F16, tag="wt")
            nc.sync.dma_start(out=wt_sb[:], in_=wt[:])
            bb_sb = pl_const.tile([4, 1], F32, tag="bb")
            nc.sync.dma_start(out=bb_sb[:], in_=bb[:])

            # rest of the x stream (DMA self-flow-controls via pool bufs)
            for g in range(2, XG):
                load_x(0, g)
            for g in range(XG):
                load_x(1, g)

            # ---- A-phase indicators for BOTH samples (consts only) ----
            ind_t = {}
            for s in range(SPC):
                for ci in range(NCH):
                    lo, hi = chunk_wbs[ci][0], chunk_wbs[ci][-1]
                    nb = hi - lo + 1
                    ind = pl_ind.tile([P, max_nb * P], F32R, tag="ind",
                                      name=f"ind_{s}_{ci}")
                    nc.vector.tensor_scalar(
                        out=ind[:, 0:nb * P],
                        in0=iota_sb[:, lo * P:(hi + 1) * P],
                        scalar1=wic_sb[:, s * NCH + ci:s * NCH + ci + 1],
                        scalar2=None,
                        op0=mybir.AluOpType.is_equal,
                    )
                    ind_t[(s, ci)] = ind

            # ---- indT builds + counts (only need wir) ----
            indT_sb = {}
            rec_sb = {}
            for s in range(SPC):
                cnt_sb = pl_small.tile([P, NST * NWB], F32, tag="cnt",
                                       name=f"cnt_{s}")
                nc.vector.memset(cnt_sb[:], 0.0)
                for si in range(NST):
                    wib = pl_wib.tile([P, STRW], F16, tag="wib",
                                      name=f"wib_{s}_{si}")
                    nc.gpsimd.partition_broadcast(
                        wib[:],
                        wir_sb[0:1, s * S + si * STRW:s * S + (si + 1) * STRW],
                    )
                    for wb in strip_wbs[si]:
                        it = pl_indT.tile([P, STRW], F16, tag="indT",
                                          name=f"indT_{s}_{si}_{wb}")
                        nc.vector.tensor_scalar(
                            out=it[:],
                            in0=wib[:],
                            scalar1=iotap_sb[:, wb:wb + 1],
                            scalar2=None,
                            op0=mybir.AluOpType.is_equal,
                            op1=mybir.AluOpType.add,
                            accum_out=cnt_sb[
                                :, si * NWB + wb:si * NWB + wb + 1],
                        )
                        indT_sb[(s, si, wb)] = it
                # counts -> reciprocals [P, NWB]
                cntw_sb = pl_small.tile([P, NWB], F32, tag="cntw",
                                        name=f"cntw_{s}")
                for wb in range(NWB):
                    nc.vector.tensor_reduce(
                        out=cntw_sb[:, wb:wb + 1],
                        in_=cnt_sb[:, wb::NWB],
                        axis=mybir.AxisListType.X,
                        op=mybir.AluOpType.add,
                    )
                rec = pl_small.tile([P, NWB], F32, tag="rec",
                                    name=f"rec_{s}")
                nc.vector.tensor_scalar_max(cntw_sb[:], cntw_sb[:], 1.0)
                nc.vector.reciprocal(rec[:], cntw_sb[:])
                rec_sb[s] = rec

            for s in range(SPC):
                # ---------------- Phase A: segment sums ----------------
                sums_ps = {}
                ymean_sb = {}
                pending_b = []

                def flush_b(s=s, ymean_sb=ymean_sb, pending_b=pending_b):
                    """B matmuls for an earlier-retired block (its XBAR
                    transpose has long finished, so no PE head-block)."""
                    while pending_b:
                        wb, sumsT = pending_b.pop(0)
                        ymr = pp_ymr.tile([P, 4], F32, tag="ymr",
                                          name=f"ymr_{s}_{wb}")
                        for hc in range(NHC):
                            nc.tensor.matmul(
                                out=ymr[:, 0:C],
                                lhsT=sumsT[:, hc * P:(hc + 1) * P],
                                rhs=wt_sb[:, hc * C:(hc + 1) * C],
                                start=(hc == 0),
                                stop=(hc == NHC - 1),
                            )
                        ym = pl_small.tile([P, 4], F16, tag="ymean",
                                           name=f"ymean_{s}_{wb}")
                        nc.vector.memset(ym[:], 0.0)
                        nc.vector.tensor_scalar(
                            out=ym[:, 0:C],
                            in0=ymr[:, 0:C],
                            scalar1=rec_sb[s][:, wb:wb + 1],
                            scalar2=None,
                            op0=mybir.AluOpType.mult,
                        )
                        ymean_sb[wb] = ym

                for ci in range(NCH):
                    x4 = x_tiles[(s, ci // XCH)]
                    xv = x4[:, (ci % XCH) * H:(ci % XCH + 1) * H]
                    lo = chunk_wbs[ci][0]
                    ind = ind_t[(s, ci)]
                    for wb in chunk_wbs[ci]:
                        if ci == first_ch[wb]:
                            sums_ps[wb] = pp_sums.tile(
                                [P, H], F32, tag="sums",
                                name=f"sums_{s}_{wb}")
                        for hh in range(2):
                            nc.tensor.matmul(
                                out=sums_ps[wb][:, hh * 512:(hh + 1) * 512],
                                lhsT=ind[:, (wb - lo) * P:(wb - lo + 1) * P],
                                rhs=xv[:, hh * 512:(hh + 1) * 512],
                                start=(ci == first_ch[wb]),
                                stop=(ci == last_ch[wb]),
                            )
                    # retire finished blocks: evac (fp16) + XBAR transpose
                    for wb in list(sums_ps.keys()):
                        if ci != last_ch[wb]:
                            continue
                        flush_b()
                        sums_sb = pl_sums.tile([P, H], F16, tag="sums_sb",
                                               name=f"sums_sb_{s}_{wb}")
                        nc.scalar.copy(out=sums_sb[:], in_=sums_ps[wb][:])
                        del sums_ps[wb]
                        sumsT = pl_sumsT.tile([P, H], F16, tag="sumsT",
                                              name=f"sumsT_{s}_{wb}")
                        nc.scalar.dma_start_transpose(
                            out=sumsT[:].rearrange("p (g w) -> p g w", g=NHC),
                            in_=sums_sb[:],
                        )
                        pending_b.append((wb, sumsT))
                flush_b()

                # ---------------- Phase C: gather ----------------------
                out_sb = pl_out.tile([4, S], F32, tag="out",
                                     name=f"out_sb_{s}")
                for si in range(NST):
                    outT = pp_out.tile([4, STRW], F32, tag="outT",
                                       name=f"outT_{s}_{si}")
                    wbs = strip_wbs[si]
                    for j, wb in enumerate(wbs):
                        for hh in range(STRW // 512):
                            nc.tensor.matmul(
                                out=outT[:, hh * 512:(hh + 1) * 512],
                                lhsT=ymean_sb[wb][:],
                                rhs=indT_sb[(s, si, wb)][
                                    :, hh * 512:(hh + 1) * 512],
                                start=(j == 0),
                                stop=(j == len(wbs) - 1),
                            )
                    nc.scalar.activation(
                        out=out_sb[:, si * STRW:(si + 1) * STRW],
                        in_=outT[:],
                        func=mybir.ActivationFunctionType.Identity,
                        bias=bb_sb[:],
                    )
                nc.scalar.dma_start(out=yout[s], in_=out_sb[0:C, :])

    nc.compile()
    return nc


def _host_inputs(x, word_ids, W, b):
    """Per-core input maps (shared by kernel() and the test's sim path)."""
    wif = word_ids.astype(np.float32)
    iota = np.broadcast_to(np.arange(NW, dtype=np.float32), (P, NW)).copy()
    iotap = (np.arange(P, dtype=np.float32)[:, None]
             + P * np.arange(NWB, dtype=np.float32)[None, :]).copy()
    wt = np.zeros((P, NHC * C), dtype=np.float16)
    for hc in range(NHC):
        wt[:, hc * C:(hc + 1) * C] = W[hc * P:(hc + 1) * P, :]
    bb = np.zeros((4, 1), dtype=np.float32)
    bb[:C, 0] = b

    in_maps = []
    for core in range(N_CORES):
        sl = slice(core * SPC, (core + 1) * SPC)
        wi_core = wif[sl]                                   # [SPC, S]
        wic = np.zeros((P, SPC * NCH), dtype=np.float32)
        for s in range(SPC):
            for ci in range(NCH):
                wic[:, s * NCH + ci] = wi_core[s, ci * P:(ci + 1) * P]
        in_maps.append({
            "xc": x[sl].reshape(SPC * S, H),
            "wic": wic,
            "wir": wi_core.reshape(1, -1).astype(np.float16),
            "iota": iota,
            "iotap": iotap,
            "wt": wt,
            "bb": bb,
        })
    return in_maps


def kernel(x, word_ids, W, b):
    x = np.ascontiguousarray(np.asarray(x, dtype=np.float32))
    word_ids = np.asarray(word_ids, dtype=np.int32)
    W = np.asarray(W, dtype=np.float32)
    b = np.asarray(b, dtype=np.float32)

    maps = _build_maps(word_ids)
    key = repr(maps)
    if key not in _CACHE:
        _CACHE[key] = _build_program(maps)
    nc = _CACHE[key]

    in_maps = _host_inputs(x, word_ids, W, b)

    global LAST_RESULTS
    res = run_bass_kernel_spmd(nc, in_maps, list(range(N_CORES)), trace=TRACE)
    LAST_RESULTS = res
    out = np.empty((B, S, C), dtype=np.float32)
    for core in range(N_CORES):
        yc = res.results[core]["yout"]                      # [SPC, C, S]
        out[core * SPC:(core + 1) * SPC] = yc.transpose(0, 2, 1)
    return out
